# revision 1
# baseline (speedup 1.0000x reference)
"""Deformable-DETR encoder layer on 8 trn2 NeuronCores (axon/jax).

Split: all dense tensor compute (q/value/offset/attn projections, softmax,
output projection, residual+LayerNorm, FFN) runs on the 8 NeuronCores via
a gather-free shard_map graph, data-parallel over (batch=2 x 4 token
chunks). The data-dependent bilinear gather-and-blend (no dense FLOPs,
pure indexed reads) runs between the two device phases in vectorized
numpy on host.

Phase 1 (device): q = src+pos; value/off/attn projections; softmax.
Host:            bilinear sample + attention-weighted reduction.
Phase 2 (device): out-proj + residual LN + FFN + LN.
"""
import functools

import jax
import jax.numpy as jnp
import numpy as np
from jax.experimental.shard_map import shard_map
from jax.sharding import Mesh, PartitionSpec as P

SHAPES = ((100, 100), (50, 50), (25, 25), (13, 13))
B, D, NH, NL, NP, DFF = 2, 256, 8, 4, 4, 1024
DH = D // NH
S = sum(h * w for h, w in SHAPES)  # 13294
NCHUNK = 4
SPAD = ((S + NCHUNK - 1) // NCHUNK) * NCHUNK  # 13296
TC = SPAD // NCHUNK

_OFFSET_NORM = np.array([[w, h] for h, w in SHAPES], np.float32)
_LVL_START = np.cumsum([0] + [h * w for h, w in SHAPES])


def _layer_norm(x, w, b):
    m = x.mean(-1, keepdims=True)
    v = ((x - m) ** 2).mean(-1, keepdims=True)
    return (x - m) * jax.lax.rsqrt(v + 1e-5) * w + b


@functools.lru_cache(maxsize=1)
def _mesh():
    devs = np.array(jax.devices()[:8]).reshape(2, 4)
    return Mesh(devs, ("b", "c"))


@functools.lru_cache(maxsize=1)
def _phase1():
    mesh = _mesh()

    def body(src, pos, w_value, b_value, w_off, b_off, w_attn, b_attn):
        # block shapes [1,1,TC,D]
        s = src[0, 0]
        q = s + pos[0, 0]
        value = s @ w_value + b_value
        off = q @ w_off + b_off
        attn = jax.nn.softmax(
            (q @ w_attn + b_attn).reshape(TC, NH, NL * NP), axis=-1
        ).reshape(TC, NH * NL * NP)
        return (value.astype(jnp.bfloat16)[None, None],
                off.astype(jnp.bfloat16)[None, None],
                attn.astype(jnp.bfloat16)[None, None])

    fn = shard_map(
        body, mesh=mesh,
        in_specs=(P("b", "c"), P("b", "c")) + (P(None),) * 6,
        out_specs=(P("b", "c"), P("b", "c"), P("b", "c")),
        check_rep=False,
    )
    return jax.jit(fn)


@functools.lru_cache(maxsize=1)
def _phase2():
    mesh = _mesh()

    def body(acc, src, w_out, b_out, w_ff1, b_ff1, w_ff2, b_ff2,
             ln1_w, ln1_b, ln2_w, ln2_b):
        a = acc[0, 0].astype(jnp.float32)
        s = src[0, 0]
        ca = a @ w_out + b_out
        x = _layer_norm(s + ca, ln1_w, ln1_b)
        ff = jax.nn.relu(x @ w_ff1 + b_ff1) @ w_ff2 + b_ff2
        return _layer_norm(x + ff, ln2_w, ln2_b)[None, None]

    fn = shard_map(
        body, mesh=mesh,
        in_specs=(P("b", "c"), P("b", "c")) + (P(None),) * 10,
        out_specs=P("b", "c"),
        check_rep=False,
    )
    return jax.jit(fn)


@functools.lru_cache(maxsize=1)
def _sample_jit():
    cpu = jax.devices("cpu")[0]

    def fn(value, off, attn, ref):
        offv = off[:, :S].reshape(B, S, NH, NL, NP, 2)
        attnv = attn[:, :S].reshape(B, S, NH, NL, NP)
        out = jnp.zeros((B, S, NH, DH), jnp.float32)
        for l, (H_, W_) in enumerate(SHAPES):
            v = value[:, _LVL_START[l]:_LVL_START[l + 1]].reshape(
                B, H_ * W_, NH, DH)
            x = ref[:, :, l, 0, None, None] * W_ - 0.5 + offv[..., l, :, 0]
            y = ref[:, :, l, 1, None, None] * H_ - 0.5 + offv[..., l, :, 1]
            x0 = jnp.floor(x)
            y0 = jnp.floor(y)
            a = attnv[..., l, :]
            for dx, dy in ((0, 0), (1, 0), (0, 1), (1, 1)):
                xi = x0 + dx
                yi = y0 + dy
                w = (1.0 - jnp.abs(x - xi)) * (1.0 - jnp.abs(y - yi)) * a
                valid = (xi >= 0) & (xi < W_) & (yi >= 0) & (yi < H_)
                w = jnp.where(valid, w, 0.0)  # [B,S,NH,NP]
                idx = (jnp.clip(yi, 0, H_ - 1) * W_
                       + jnp.clip(xi, 0, W_ - 1)).astype(jnp.int32)
                idx_t = idx.transpose(0, 1, 3, 2).reshape(B, -1, NH, 1)
                g = jnp.take_along_axis(v, idx_t, axis=1).reshape(
                    B, S, NP, NH, DH)
                out = out + (g * w.transpose(0, 1, 3, 2)[..., None]).sum(2)
        return out.reshape(B, S, D)

    return jax.jit(fn, device=cpu)


def _sample_host(value, off, attn, ref):
    return np.asarray(_sample_jit()(value, off, attn, ref))


def kernel(**inputs):
    f32 = lambda k: np.asarray(inputs[k], np.float32)
    src, pos, ref = f32("src"), f32("pos"), f32("reference_points")

    pad = SPAD - S
    pad_tok = lambda a: np.concatenate(
        [a, np.zeros((B, pad) + a.shape[2:], a.dtype)], 1)
    from jax.sharding import NamedSharding
    mesh = _mesh()
    sh = NamedSharding(mesh, P("b", "c"))
    src_p = jax.device_put(pad_tok(src).reshape(B, NCHUNK, TC, D), sh)
    pos_p = jax.device_put(pad_tok(pos).reshape(B, NCHUNK, TC, D), sh)

    value, off, attn = _phase1()(
        src_p, pos_p, f32("w_value"), f32("b_value"),
        f32("w_off"), f32("b_off"), f32("w_attn"), f32("b_attn"))
    value = np.asarray(value).astype(np.float32).reshape(B, SPAD, D)[:, :S]
    off = np.asarray(off).astype(np.float32).reshape(B, SPAD, NH * NL * NP * 2)
    attn = np.asarray(attn).astype(np.float32).reshape(B, SPAD, NH * NL * NP)

    acc = _sample_host(value, off, attn, ref)

    acc_p = jax.device_put(
        pad_tok(acc).reshape(B, NCHUNK, TC, D).astype(jnp.bfloat16), sh)
    out = _phase2()(
        acc_p, src_p, f32("w_out"), f32("b_out"), f32("w_ff1"), f32("b_ff1"),
        f32("w_ff2"), f32("b_ff2"), f32("ln1_w"), f32("ln1_b"),
        f32("ln2_w"), f32("ln2_b"))
    return np.asarray(out).reshape(B, SPAD, D)[:, :S]



# revision 4
# speedup vs baseline: 5.7723x; 5.7723x over previous
"""Deformable-DETR encoder layer, fused + pipelined on 8 trn2 NeuronCores.

Same on-device math as v2 (all projections, softmax, bilinear gather,
FFN, LayerNorms on device; bf16/int8 wire), but restructured into a
value-projection program plus G per-token-group stage programs so that
uploads, device execution, and downloads overlap on the slow (~55 MB/s)
axon tunnel:

  wire up:   wmat bf16 (sharded) | small f32 | src bf16 | per-group pos
             int8 + ref f32
  device:    value = src@Wv + all-gather (once) ; per group: q/off/attn,
             gather-blend, out-proj, LN, FFN, LN
  wire down: per-group out bf16, fetched async while later groups run
"""
import functools

import jax
import jax.numpy as jnp
import numpy as np
from jax.experimental.shard_map import shard_map
from jax.sharding import Mesh, NamedSharding, PartitionSpec as P

SHAPES = ((100, 100), (50, 50), (25, 25), (13, 13))
B, D, NH, NL, NP, DFF = 2, 256, 8, 4, 4, 1024
DH = D // NH
S = sum(h * w for h, w in SHAPES)  # 13294
NCHUNK = 4
G = 4                              # pipeline groups per core
SC = ((S + NCHUNK * G - 1) // (NCHUNK * G)) * G  # per-core tokens, mult of G
SPAD = SC * NCHUNK                 # padded per-batch tokens
SG = SC // G                       # tokens per core per group

_LVL_START = np.cumsum([0] + [h * w for h, w in SHAPES])

_WSEG = {}
_off = 0
for _name, _shape in (("w_value", (D, D)), ("w_off", (D, D)), ("w_attn", (D, NH * NL * NP)),
                      ("w_out", (D, D)), ("w_ff1", (D, DFF)), ("w_ff2", (DFF, D))):
    _WSEG[_name] = (_off, _shape)
    _off += _shape[0] * _shape[1]
_WTOT = _off
assert _WTOT % 8 == 0

_BSEG = {}
_off = 0
for _name, _n in (("b_value", D), ("b_off", D), ("b_attn", NH * NL * NP), ("b_out", D),
                  ("b_ff1", DFF), ("b_ff2", D), ("ln1_w", D), ("ln1_b", D),
                  ("ln2_w", D), ("ln2_b", D), ("s_pos", 1)):
    _BSEG[_name] = (_off, _n)
    _off += _n

_BF = jnp.bfloat16
_F32 = jnp.float32


@functools.lru_cache(maxsize=1)
def _mesh():
    devs = np.array(jax.devices()[:8]).reshape(2, 4)
    return Mesh(devs, ("b", "c"))


def _ln(x, w, b):
    m = x.mean(-1, keepdims=True)
    v = ((x - m) ** 2).mean(-1, keepdims=True)
    return (x - m) * jax.lax.rsqrt(v + 1e-5) * w + b


def _wseg(wall, name):
    o, shp = _WSEG[name]
    return wall[o:o + shp[0] * shp[1]].reshape(shp)


def _bseg(small, name):
    o, n = _BSEG[name]
    return small[o:o + n]


@functools.lru_cache(maxsize=1)
def _value_prog():
    mesh = _mesh()

    def body(*args):
        # args: G src groups [1,1,SG,D] bf16, wmat [1,1,_WTOT//8] bf16, small [_BTOT] f32
        srcs, wmat, small = args[:G], args[G], args[G + 1]
        wall_c = jax.lax.all_gather(wmat[0, 0], "c", axis=0)
        wall = jax.lax.all_gather(wall_c, "b", axis=0).reshape(-1)
        w_v = _wseg(wall, "w_value")
        src_c = jnp.concatenate([s[0, 0] for s in srcs], 0)   # [SC, D]
        v = jnp.dot(src_c, w_v, preferred_element_type=_F32)
        v = (v + _bseg(small, "b_value")).astype(_BF)
        vg = jax.lax.all_gather(v, "c", axis=0, tiled=True)   # [SPAD, D]
        return wall[None], vg[None]

    fn = shard_map(
        body, mesh=mesh,
        in_specs=(P("b", "c"),) * G + (P("b", "c"), P(None)),
        out_specs=(P(None), P("b")),
        check_rep=False,
    )
    return jax.jit(fn)


@functools.lru_cache(maxsize=1)
def _stage_prog():
    mesh = _mesh()

    def body(vg, wall, src, pos, ref, small):
        # vg [1, SPAD, D] bf16 (replicated over c); wall [1, _WTOT] bf16 (replicated);
        # src/pos [1,1,SG,D]; ref [1,1,SG,NL,2]
        wall = wall[0]
        vg = vg[0].reshape(SPAD, NH, DH)
        s_ = src[0, 0]
        s_pos = _bseg(small, "s_pos")[0]
        q = (s_.astype(_F32) + pos[0, 0].astype(_F32) * s_pos).astype(_BF)
        mm = lambda a, w: jnp.dot(a, w, preferred_element_type=_F32)
        off = (mm(q, _wseg(wall, "w_off")) + _bseg(small, "b_off")).reshape(SG, NH, NL, NP, 2)
        logits = (mm(q, _wseg(wall, "w_attn")) + _bseg(small, "b_attn")).reshape(SG, NH, NL * NP)
        attn = jax.nn.softmax(logits, axis=-1).reshape(SG, NH, NL, NP)

        r = ref[0, 0]
        acc = jnp.zeros((SG, NH, DH), _F32)
        for l, (H_, W_) in enumerate(SHAPES):
            x = r[:, l, 0][:, None, None] * W_ - 0.5 + off[:, :, l, :, 0]
            y = r[:, l, 1][:, None, None] * H_ - 0.5 + off[:, :, l, :, 1]
            x0 = jnp.floor(x)
            y0 = jnp.floor(y)
            a = attn[:, :, l, :]
            idxs, wgts = [], []
            for dx, dy in ((0, 0), (1, 0), (0, 1), (1, 1)):
                xi = x0 + dx
                yi = y0 + dy
                w = (1.0 - jnp.abs(x - xi)) * (1.0 - jnp.abs(y - yi)) * a
                valid = (xi >= 0) & (xi < W_) & (yi >= 0) & (yi < H_)
                wgts.append(jnp.where(valid, w, 0.0))
                idxs.append((_LVL_START[l] + jnp.clip(yi, 0, H_ - 1) * W_
                             + jnp.clip(xi, 0, W_ - 1)).astype(jnp.int32))
            idx = jnp.stack(idxs, 2)                      # [SG, NH, 4, NP]
            wgt = jnp.stack(wgts, 2)
            idx_t = idx.transpose(0, 2, 3, 1).reshape(SG * 4 * NP, NH, 1)
            wgt_t = wgt.transpose(0, 2, 3, 1).reshape(SG * 4 * NP, NH, 1)
            g = jnp.take_along_axis(vg, idx_t, axis=0, mode="promise_in_bounds")
            acc = acc + (g.astype(_F32) * wgt_t).reshape(SG, 4 * NP, NH, DH).sum(1)

        ca = mm(acc.reshape(SG, D).astype(_BF), _wseg(wall, "w_out")) + _bseg(small, "b_out")
        x1 = _ln(s_.astype(_F32) + ca, _bseg(small, "ln1_w"), _bseg(small, "ln1_b"))
        h = jax.nn.relu(mm(x1.astype(_BF), _wseg(wall, "w_ff1")) + _bseg(small, "b_ff1"))
        ff = mm(h.astype(_BF), _wseg(wall, "w_ff2")) + _bseg(small, "b_ff2")
        out = _ln(x1 + ff, _bseg(small, "ln2_w"), _bseg(small, "ln2_b"))
        return out.astype(_BF)[None, None]

    fn = shard_map(
        body, mesh=mesh,
        in_specs=(P("b"), P(None), P("b", "c"), P("b", "c"), P("b", "c"), P(None)),
        out_specs=P("b", "c"),
        check_rep=False,
    )
    return jax.jit(fn)


def _pad_tok(a):
    pad = SPAD - S
    if pad == 0:
        return a
    return np.concatenate([a, np.zeros((B, pad) + a.shape[2:], a.dtype)], 1)


def kernel(**inputs):
    f32 = lambda k: np.asarray(inputs[k], np.float32)
    mesh = _mesh()
    sh = NamedSharding(mesh, P("b", "c"))
    sh_rep = NamedSharding(mesh, P(None))

    # --- weights + src first (value projection is the pipeline head)
    src = f32("src")
    wmat = np.concatenate([
        np.asarray(inputs[name], np.float32).ravel() for name in
        ("w_value", "w_off", "w_attn", "w_out", "w_ff1", "w_ff2")
    ]).astype(_BF).reshape(B, NCHUNK, _WTOT // 8)
    wmat_d = jax.device_put(wmat, sh)

    pos = f32("pos")
    s_pos = float(np.abs(pos).max()) / 127.0
    if s_pos == 0.0:
        s_pos = 1.0
    small = np.concatenate([
        np.asarray(inputs[name], np.float32).ravel() for name in
        ("b_value", "b_off", "b_attn", "b_out", "b_ff1", "b_ff2",
         "ln1_w", "ln1_b", "ln2_w", "ln2_b")
    ] + [np.array([s_pos], np.float32)])
    small_d = jax.device_put(small, sh_rep)

    src_h = _pad_tok(src).reshape(B, NCHUNK, G, SG, D).astype(_BF)
    src_gd = [jax.device_put(np.ascontiguousarray(src_h[:, :, g]), sh)
              for g in range(G)]

    wall_d, vg_d = _value_prog()(*src_gd, wmat_d, small_d)

    pos_h = np.clip(np.rint(pos * (1.0 / s_pos)), -127, 127).astype(np.int8)
    pos_h = _pad_tok(pos_h).reshape(B, NCHUNK, G, SG, D)
    ref_h = _pad_tok(f32("reference_points")).reshape(B, NCHUNK, G, SG, NL, 2)

    stage = _stage_prog()
    outs = []
    for g in range(G):
        pos_d = jax.device_put(np.ascontiguousarray(pos_h[:, :, g]), sh)
        ref_d = jax.device_put(np.ascontiguousarray(ref_h[:, :, g]), sh)
        o = stage(vg_d, wall_d, src_gd[g], pos_d, ref_d, small_d)
        o.copy_to_host_async()
        outs.append(o)

    out = np.stack([np.asarray(o) for o in outs], 2)  # [B, NCHUNK, G, SG, D]
    return out.astype(np.float32).reshape(B, SPAD, D)[:, :S]


# revision 6
# speedup vs baseline: 6.2116x; 1.0761x over previous
"""Deformable-DETR encoder layer, fused + pipelined on 8 trn2 NeuronCores.

All compute runs on-device (projections, softmax, bilinear gather-blend,
output projection, LayerNorms, FFN). The axon tunnel is half-duplex at
~55 MB/s, so total wire bytes dominate; the kernel minimizes them:

  up:   src bf16 (13.6MB) | pos int4-packed (3.4MB) | ref uint16 (0.43MB)
        | weights bf16 sharded (1.5MB, all-gathered on device) | biases f32
  down: pre-affine LN2 output as int8 (6.8MB, fixed 3.9-sigma scale);
        the final `z*ln2_w + ln2_b` affine is applied on host in f32.

Pipeline: an early program all-gathers the sharded weight upload; a value
program computes value = src@Wv and all-gathers it within each batch
group; G per-token-group stage programs then overlap device execution
with the pos upload and the int8 output downloads.

Sharding: mesh (b=2, c=4) — data-parallel over batch, token-parallel
within a batch group; the flattened multi-scale value memory is
replicated within the group by the on-device all-gather.
"""
import functools

import jax
import jax.numpy as jnp
import numpy as np
from jax.experimental.shard_map import shard_map
from jax.sharding import Mesh, NamedSharding, PartitionSpec as P

SHAPES = ((100, 100), (50, 50), (25, 25), (13, 13))
B, D, NH, NL, NP, DFF = 2, 256, 8, 4, 4, 1024
DH = D // NH
S = sum(h * w for h, w in SHAPES)  # 13294
NCHUNK = 4
G = 4                              # pipeline groups per core
SC = ((S + NCHUNK * G - 1) // (NCHUNK * G)) * G  # 3324 per-core tokens
SPAD = SC * NCHUNK                 # 13296 padded per-batch tokens
SG = SC // G                       # 831 tokens per core per group

Z_CLIP = 3.9                       # int8 clip for the unit-variance LN output
Z_SCALE = 127.0 / Z_CLIP

_LVL_START = np.cumsum([0] + [h * w for h, w in SHAPES])

_WSEG = {}
_off = 0
for _name, _shape in (("w_value", (D, D)), ("w_off", (D, D)), ("w_attn", (D, NH * NL * NP)),
                      ("w_out", (D, D)), ("w_ff1", (D, DFF)), ("w_ff2", (DFF, D))):
    _WSEG[_name] = (_off, _shape)
    _off += _shape[0] * _shape[1]
_WTOT = _off
assert _WTOT % 8 == 0

_BSEG = {}
_off = 0
for _name, _n in (("b_value", D), ("b_off", D), ("b_attn", NH * NL * NP), ("b_out", D),
                  ("b_ff1", DFF), ("b_ff2", D), ("ln1_w", D), ("ln1_b", D),
                  ("s_pos", 1)):
    _BSEG[_name] = (_off, _n)
    _off += _n
_BTOT = _off

_BF = jnp.bfloat16
_F32 = jnp.float32


@functools.lru_cache(maxsize=1)
def _mesh():
    devs = np.array(jax.devices()[:8]).reshape(2, 4)
    return Mesh(devs, ("b", "c"))


def _wseg(wall, name):
    o, shp = _WSEG[name]
    return wall[o:o + shp[0] * shp[1]].reshape(shp)


def _bseg(small, name):
    o, n = _BSEG[name]
    return small[o:o + n]


@functools.lru_cache(maxsize=1)
def _wgather_prog():
    mesh = _mesh()

    def body(wmat):
        wall_c = jax.lax.all_gather(wmat[0, 0], "c", axis=0)
        wall = jax.lax.all_gather(wall_c, "b", axis=0).reshape(-1)
        return wall[None]

    fn = shard_map(body, mesh=mesh, in_specs=(P("b", "c"),),
                   out_specs=P(None), check_rep=False)
    return jax.jit(fn)


@functools.lru_cache(maxsize=1)
def _value_prog():
    mesh = _mesh()

    def body(*args):
        # args: G src groups [1,1,SG,D] bf16, wall [1,_WTOT] bf16, small [_BTOT] f32
        srcs, wall, small = args[:G], args[G][0], args[G + 1]
        src_c = jnp.concatenate([s[0, 0] for s in srcs], 0)   # [SC, D]
        v = jnp.dot(src_c, _wseg(wall, "w_value"), preferred_element_type=_F32)
        v = (v + _bseg(small, "b_value")).astype(_BF)
        vg = jax.lax.all_gather(v, "c", axis=0, tiled=True)   # [SPAD, D]
        return vg[None]

    fn = shard_map(
        body, mesh=mesh,
        in_specs=(P("b", "c"),) * G + (P(None), P(None)),
        out_specs=P("b"),
        check_rep=False,
    )
    return jax.jit(fn)


@functools.lru_cache(maxsize=1)
def _stage_prog():
    mesh = _mesh()

    def body(vg, wall, src, pos, ref, small):
        # vg [1,SPAD,D] bf16 (replicated over c); wall [1,_WTOT] bf16;
        # src [1,1,SG,D] bf16; pos [1,1,SG,D//2] uint8 (int4 pairs);
        # ref [1,1,SG,NL,2] uint16
        wall = wall[0]
        vg = vg[0].reshape(SPAD, NH, DH)
        s_ = src[0, 0]
        s_pos = _bseg(small, "s_pos")[0]
        u = pos[0, 0]
        hi = (jnp.right_shift(u, 4)).astype(_F32) - 8.0
        lo = (jnp.bitwise_and(u, 15)).astype(_F32) - 8.0
        pq = jnp.stack([hi, lo], -1).reshape(SG, D) * s_pos
        q = (s_.astype(_F32) + pq).astype(_BF)
        mm = lambda a, w: jnp.dot(a, w, preferred_element_type=_F32)
        off = (mm(q, _wseg(wall, "w_off")) + _bseg(small, "b_off")).reshape(SG, NH, NL, NP, 2)
        logits = (mm(q, _wseg(wall, "w_attn")) + _bseg(small, "b_attn")).reshape(SG, NH, NL * NP)
        attn = jax.nn.softmax(logits, axis=-1).reshape(SG, NH, NL, NP)

        r = ref[0, 0].astype(_F32) * (1.0 / 65535.0)          # [SG, NL, 2]
        acc = jnp.zeros((SG, NH, DH), _F32)
        for l, (H_, W_) in enumerate(SHAPES):
            x = r[:, l, 0][:, None, None] * W_ - 0.5 + off[:, :, l, :, 0]
            y = r[:, l, 1][:, None, None] * H_ - 0.5 + off[:, :, l, :, 1]
            x0 = jnp.floor(x)
            y0 = jnp.floor(y)
            a = attn[:, :, l, :]
            idxs, wgts = [], []
            for dx, dy in ((0, 0), (1, 0), (0, 1), (1, 1)):
                xi = x0 + dx
                yi = y0 + dy
                w = (1.0 - jnp.abs(x - xi)) * (1.0 - jnp.abs(y - yi)) * a
                valid = (xi >= 0) & (xi < W_) & (yi >= 0) & (yi < H_)
                wgts.append(jnp.where(valid, w, 0.0))
                idxs.append((_LVL_START[l] + jnp.clip(yi, 0, H_ - 1) * W_
                             + jnp.clip(xi, 0, W_ - 1)).astype(jnp.int32))
            idx = jnp.stack(idxs, 2)                      # [SG, NH, 4, NP]
            wgt = jnp.stack(wgts, 2)
            idx_t = idx.transpose(0, 2, 3, 1).reshape(SG * 4 * NP, NH, 1)
            wgt_t = wgt.transpose(0, 2, 3, 1).reshape(SG * 4 * NP, NH, 1)
            g = jnp.take_along_axis(vg, idx_t, axis=0, mode="promise_in_bounds")
            acc = acc + (g.astype(_F32) * wgt_t).reshape(SG, 4 * NP, NH, DH).sum(1)

        ca = mm(acc.reshape(SG, D).astype(_BF), _wseg(wall, "w_out")) + _bseg(small, "b_out")
        x = s_.astype(_F32) + ca
        m = x.mean(-1, keepdims=True)
        v = ((x - m) ** 2).mean(-1, keepdims=True)
        x1 = (x - m) * jax.lax.rsqrt(v + 1e-5) * _bseg(small, "ln1_w") + _bseg(small, "ln1_b")
        h = jax.nn.relu(mm(x1.astype(_BF), _wseg(wall, "w_ff1")) + _bseg(small, "b_ff1"))
        ff = mm(h.astype(_BF), _wseg(wall, "w_ff2")) + _bseg(small, "b_ff2")
        y2 = x1 + ff
        m2 = y2.mean(-1, keepdims=True)
        v2 = ((y2 - m2) ** 2).mean(-1, keepdims=True)
        z = (y2 - m2) * jax.lax.rsqrt(v2 + 1e-5)              # pre-affine LN output
        zq = jnp.rint(jnp.clip(z, -Z_CLIP, Z_CLIP) * Z_SCALE).astype(jnp.int8)
        return zq[None, None]

    fn = shard_map(
        body, mesh=mesh,
        in_specs=(P("b"), P(None), P("b", "c"), P("b", "c"), P("b", "c"), P(None)),
        out_specs=P("b", "c"),
        check_rep=False,
    )
    return jax.jit(fn)


def _pad_tok(a):
    pad = SPAD - S
    if pad == 0:
        return a
    return np.concatenate([a, np.zeros((B, pad) + a.shape[2:], a.dtype)], 1)


def kernel(**inputs):
    f32 = lambda k: np.asarray(inputs[k], np.float32)
    mesh = _mesh()
    sh = NamedSharding(mesh, P("b", "c"))
    sh_rep = NamedSharding(mesh, P(None))

    # 1. weights first (small; the gather program runs while src uploads)
    wmat = np.concatenate([
        np.asarray(inputs[name], np.float32).ravel() for name in
        ("w_value", "w_off", "w_attn", "w_out", "w_ff1", "w_ff2")
    ]).astype(_BF).reshape(B, NCHUNK, _WTOT // 8)
    wmat_d = jax.device_put(wmat, sh)
    wall_d = _wgather_prog()(wmat_d)

    # 2. src groups on the wire as early as possible (value is the head)
    src = f32("src")
    src_h = _pad_tok(src.astype(_BF)).reshape(B, NCHUNK, G, SG, D)
    src_gd = [jax.device_put(np.ascontiguousarray(src_h[:, :, g]), sh)
              for g in range(G)]

    pos = f32("pos")
    s_pos = float(np.abs(pos).max()) / 7.0
    if s_pos == 0.0:
        s_pos = 1.0
    small = np.concatenate([
        np.asarray(inputs[name], np.float32).ravel() for name in
        ("b_value", "b_off", "b_attn", "b_out", "b_ff1", "b_ff2",
         "ln1_w", "ln1_b")
    ] + [np.array([s_pos], np.float32)])
    small_d = jax.device_put(small, sh_rep)

    vg_d = _value_prog()(*src_gd, wall_d, small_d)

    # 3. ref (small, needed by stage 0) then pos groups
    ref_h = np.rint(np.clip(f32("reference_points"), 0.0, 1.0) * 65535.0).astype(np.uint16)
    ref_h = _pad_tok(ref_h).reshape(B, NCHUNK, G, SG, NL, 2)
    ref_gd = [jax.device_put(np.ascontiguousarray(ref_h[:, :, g]), sh)
              for g in range(G)]

    q4 = (np.clip(np.rint(pos * (1.0 / s_pos)), -7, 7) + 8.0).astype(np.uint8)
    packed = (q4[..., 0::2] << 4) | q4[..., 1::2]
    pos_h = _pad_tok(packed).reshape(B, NCHUNK, G, SG, D // 2)

    stage = _stage_prog()
    outs = []
    for g in range(G):
        pos_d = jax.device_put(np.ascontiguousarray(pos_h[:, :, g]), sh)
        o = stage(vg_d, wall_d, src_gd[g], pos_d, ref_gd[g], small_d)
        o.copy_to_host_async()
        outs.append(o)

    # 4. fetch int8 z, apply the LN2 affine on host in f32
    z = np.stack([np.asarray(o) for o in outs], 2).astype(np.float32)
    z *= (Z_CLIP / 127.0)
    out = z.reshape(B, SPAD, D)[:, :S]
    out = out * f32("ln2_w") + f32("ln2_b")
    return out


# revision 10
# speedup vs baseline: 7.3932x; 1.1902x over previous
"""Deformable-DETR encoder layer, fused + pipelined on 8 trn2 NeuronCores.

All compute runs on-device (projections, softmax, bilinear gather-blend,
output projection, LayerNorms, FFN). The axon tunnel is half-duplex at
~55 MB/s, so total wire bytes dominate; the kernel minimizes them:

  up:   src bf16 (13.6MB) | per-group [pos int4-packed | ref uint16] as
        one uint8 buffer (3.9MB) | weights bf16 sharded (1.5MB,
        all-gathered on device) | biases f32
  down: pre-affine LN2 output as int8 (6.8MB, fixed 3.9-sigma scale);
        the final `z*ln2_w + ln2_b` affine is applied on host in f32.

Pipeline: a tiny program all-gathers the sharded weight upload while src
streams; stage 0 computes value = src@Wv, all-gathers it within each
batch group, and processes the first token group; stages 1..G-1 reuse
the gathered value, overlapping execution with the pos uploads and the
int8 output downloads.

Sharding: mesh (b=2, c=4) — data-parallel over batch, token-parallel
within a batch group; the flattened multi-scale value memory is
replicated within the group by the on-device all-gather.
"""
import functools

import jax
import jax.numpy as jnp
import numpy as np
from jax.experimental.shard_map import shard_map
from jax.sharding import Mesh, NamedSharding, PartitionSpec as P

SHAPES = ((100, 100), (50, 50), (25, 25), (13, 13))
B, D, NH, NL, NP, DFF = 2, 256, 8, 4, 4, 1024
DH = D // NH
S = sum(h * w for h, w in SHAPES)  # 13294
NCHUNK = 4
G = 4                              # pipeline groups per core
SC = ((S + NCHUNK * G - 1) // (NCHUNK * G)) * G  # 3324 per-core tokens
SPAD = SC * NCHUNK                 # 13296 padded per-batch tokens
SG = SC // G                       # 831 tokens per core per group

Z_CLIP = 3.9                       # int8 clip for the unit-variance LN output
Z_SCALE = 127.0 / Z_CLIP

# per-group aux buffer: [pos int4 packed D//2 | ref hi | ref lo] bytes per token
_POS_B = D // 2
_REF_B = NL * 2
_AUX_B = _POS_B + 2 * _REF_B

_LVL_START = np.cumsum([0] + [h * w for h, w in SHAPES])

_WSEG = {}
_off = 0
for _name, _shape in (("w_value", (D, D)), ("w_off", (D, D)), ("w_attn", (D, NH * NL * NP)),
                      ("w_out", (D, D)), ("w_ff1", (D, DFF)), ("w_ff2", (DFF, D))):
    _WSEG[_name] = (_off, _shape)
    _off += _shape[0] * _shape[1]
_WTOT = _off
assert _WTOT % 8 == 0

_BSEG = {}
_off = 0
for _name, _n in (("b_value", D), ("b_off", D), ("b_attn", NH * NL * NP), ("b_out", D),
                  ("b_ff1", DFF), ("b_ff2", D), ("ln1_w", D), ("ln1_b", D),
                  ("s_pos", 1)):
    _BSEG[_name] = (_off, _n)
    _off += _n
_BTOT = _off

_BF = jnp.bfloat16
_F32 = jnp.float32


@functools.lru_cache(maxsize=1)
def _mesh():
    devs = np.array(jax.devices()[:8]).reshape(2, 4)
    return Mesh(devs, ("b", "c"))


def _wseg(wall, name):
    o, shp = _WSEG[name]
    return wall[o:o + shp[0] * shp[1]].reshape(shp)


def _bseg(small, name):
    o, n = _BSEG[name]
    return small[o:o + n]


@functools.lru_cache(maxsize=1)
def _wgather_prog():
    mesh = _mesh()

    def body(wmat):
        wall_c = jax.lax.all_gather(wmat[0, 0], "c", axis=0)
        wall = jax.lax.all_gather(wall_c, "b", axis=0).reshape(-1)
        return wall[None]

    fn = shard_map(body, mesh=mesh, in_specs=(P("b", "c"),),
                   out_specs=P(None), check_rep=False)
    return jax.jit(fn)


def _group_body(vg, wall, small, s_, aux):
    """Per-group compute: q/off/attn, bilinear gather-blend, out-proj,
    LN1, FFN, pre-affine LN2 -> int8. All inputs are per-core blocks."""
    s_pos = _bseg(small, "s_pos")[0]
    u = aux[:, :_POS_B]
    hi = (jnp.right_shift(u, 4)).astype(_F32) - 8.0
    lo = (jnp.bitwise_and(u, 15)).astype(_F32) - 8.0
    pq = jnp.stack([hi, lo], -1).reshape(SG, D) * s_pos
    rhi = aux[:, _POS_B:_POS_B + _REF_B].astype(_F32)
    rlo = aux[:, _POS_B + _REF_B:].astype(_F32)
    r = ((rhi * 256.0 + rlo) * (1.0 / 65535.0)).reshape(SG, NL, 2)

    q = (s_.astype(_F32) + pq).astype(_BF)
    mm = lambda a, w: jnp.dot(a, w, preferred_element_type=_F32)
    off = (mm(q, _wseg(wall, "w_off")) + _bseg(small, "b_off")).reshape(SG, NH, NL, NP, 2)
    logits = (mm(q, _wseg(wall, "w_attn")) + _bseg(small, "b_attn")).reshape(SG, NH, NL * NP)
    attn = jax.nn.softmax(logits, axis=-1).reshape(SG, NH, NL, NP)

    acc = jnp.zeros((SG, NH, DH), _F32)
    for l, (H_, W_) in enumerate(SHAPES):
        x = r[:, l, 0][:, None, None] * W_ - 0.5 + off[:, :, l, :, 0]
        y = r[:, l, 1][:, None, None] * H_ - 0.5 + off[:, :, l, :, 1]
        x0 = jnp.floor(x)
        y0 = jnp.floor(y)
        a = attn[:, :, l, :]
        idxs, wgts = [], []
        for dx, dy in ((0, 0), (1, 0), (0, 1), (1, 1)):
            xi = x0 + dx
            yi = y0 + dy
            w = (1.0 - jnp.abs(x - xi)) * (1.0 - jnp.abs(y - yi)) * a
            valid = (xi >= 0) & (xi < W_) & (yi >= 0) & (yi < H_)
            wgts.append(jnp.where(valid, w, 0.0))
            idxs.append((_LVL_START[l] + jnp.clip(yi, 0, H_ - 1) * W_
                         + jnp.clip(xi, 0, W_ - 1)).astype(jnp.int32))
        idx = jnp.stack(idxs, 2)                      # [SG, NH, 4, NP]
        wgt = jnp.stack(wgts, 2)
        idx_t = idx.transpose(0, 2, 3, 1).reshape(SG * 4 * NP, NH, 1)
        wgt_t = wgt.transpose(0, 2, 3, 1).reshape(SG * 4 * NP, NH, 1)
        g = jnp.take_along_axis(vg, idx_t, axis=0, mode="promise_in_bounds")
        acc = acc + (g.astype(_F32) * wgt_t).reshape(SG, 4 * NP, NH, DH).sum(1)

    ca = mm(acc.reshape(SG, D).astype(_BF), _wseg(wall, "w_out")) + _bseg(small, "b_out")
    x = s_.astype(_F32) + ca
    m = x.mean(-1, keepdims=True)
    v = ((x - m) ** 2).mean(-1, keepdims=True)
    x1 = (x - m) * jax.lax.rsqrt(v + 1e-5) * _bseg(small, "ln1_w") + _bseg(small, "ln1_b")
    h = jax.nn.relu(mm(x1.astype(_BF), _wseg(wall, "w_ff1")) + _bseg(small, "b_ff1"))
    ff = mm(h.astype(_BF), _wseg(wall, "w_ff2")) + _bseg(small, "b_ff2")
    y2 = x1 + ff
    m2 = y2.mean(-1, keepdims=True)
    v2 = ((y2 - m2) ** 2).mean(-1, keepdims=True)
    z = (y2 - m2) * jax.lax.rsqrt(v2 + 1e-5)          # pre-affine LN2 output
    return jnp.rint(jnp.clip(z, -Z_CLIP, Z_CLIP) * Z_SCALE).astype(jnp.int8)


@functools.lru_cache(maxsize=1)
def _stage0_prog():
    """Value projection + all-gather + group-0 compute in one program."""
    mesh = _mesh()

    def body(*args):
        # args: G src groups [1,1,SG,D] bf16, wall [1,_WTOT] bf16,
        # small [_BTOT] f32, aux0 [1,1,SG,_AUX_B] uint8
        srcs, wall, small, aux = args[:G], args[G][0], args[G + 1], args[G + 2]
        src_c = jnp.concatenate([s[0, 0] for s in srcs], 0)   # [SC, D]
        v = jnp.dot(src_c, _wseg(wall, "w_value"), preferred_element_type=_F32)
        v = (v + _bseg(small, "b_value")).astype(_BF)
        vg = jax.lax.all_gather(v, "c", axis=0, tiled=True)   # [SPAD, D]
        z = _group_body(vg.reshape(SPAD, NH, DH), wall, small,
                        srcs[0][0, 0], aux[0, 0])
        return vg[None], z[None, None]

    fn = shard_map(
        body, mesh=mesh,
        in_specs=(P("b", "c"),) * G + (P(None), P(None), P("b", "c")),
        out_specs=(P("b"), P("b", "c")),
        check_rep=False,
    )
    return jax.jit(fn)


@functools.lru_cache(maxsize=1)
def _stage_prog():
    mesh = _mesh()

    def body(vg, wall, src, aux, small):
        z = _group_body(vg[0].reshape(SPAD, NH, DH), wall[0], small,
                        src[0, 0], aux[0, 0])
        return z[None, None]

    fn = shard_map(
        body, mesh=mesh,
        in_specs=(P("b"), P(None), P("b", "c"), P("b", "c"), P(None)),
        out_specs=P("b", "c"),
        check_rep=False,
    )
    return jax.jit(fn)


def _pad_tok(a):
    pad = SPAD - S
    if pad == 0:
        return a
    return np.concatenate([a, np.zeros((B, pad) + a.shape[2:], a.dtype)], 1)


def kernel(**inputs):
    f32 = lambda k: np.asarray(inputs[k], np.float32)
    mesh = _mesh()
    sh = NamedSharding(mesh, P("b", "c"))
    sh_rep = NamedSharding(mesh, P(None))

    # 1. weights first (small; gather program overlaps the src upload)
    wmat = np.concatenate([
        np.asarray(inputs[name], np.float32).ravel() for name in
        ("w_value", "w_off", "w_attn", "w_out", "w_ff1", "w_ff2")
    ]).astype(_BF).reshape(B, NCHUNK, _WTOT // 8)
    wmat_d = jax.device_put(wmat, sh)
    wall_d = _wgather_prog()(wmat_d)

    # 2. src groups on the wire as early as possible; core c group g owns
    # batch rows [c*SC + g*SG, c*SC + (g+1)*SG) so the stage-0 concat over
    # groups rebuilds each core's contiguous slice and the all-gather
    # rebuilds raster order
    src_h = _pad_tok(f32("src").astype(_BF)).reshape(B, NCHUNK, G, SG, D)
    src_gd = [jax.device_put(np.ascontiguousarray(src_h[:, :, g]), sh)
              for g in range(G)]

    pos = f32("pos")
    s_pos = float(np.abs(pos).max()) / 7.0
    if s_pos == 0.0:
        s_pos = 1.0
    small = np.concatenate([
        np.asarray(inputs[name], np.float32).ravel() for name in
        ("b_value", "b_off", "b_attn", "b_out", "b_ff1", "b_ff2",
         "ln1_w", "ln1_b")
    ] + [np.array([s_pos], np.float32)])
    small_d = jax.device_put(small, sh_rep)

    # 3. per-group aux = [pos int4 | ref hi | ref lo] uint8
    q4 = (np.clip(np.rint(pos * (1.0 / s_pos)), -7, 7) + 8.0).astype(np.uint8)
    packed = (q4[..., 0::2] << 4) | q4[..., 1::2]          # [B, S, D//2]
    ref_u = np.rint(np.clip(f32("reference_points"), 0.0, 1.0) * 65535.0)
    ref_u = ref_u.astype(np.uint16).reshape(B, S, _REF_B)
    aux = np.concatenate(
        [packed, (ref_u >> 8).astype(np.uint8), (ref_u & 255).astype(np.uint8)],
        axis=2)                                             # [B, S, _AUX_B]
    aux = _pad_tok(aux).reshape(B, NCHUNK, G, SG, _AUX_B)

    aux0_d = jax.device_put(np.ascontiguousarray(aux[:, :, 0]), sh)

    vg_d, z0_d = _stage0_prog()(*src_gd, wall_d, small_d, aux0_d)
    z0_d.copy_to_host_async()
    outs = [z0_d]

    stage = _stage_prog()
    for g in range(1, G):
        aux_d = jax.device_put(np.ascontiguousarray(aux[:, :, g]), sh)
        o = stage(vg_d, wall_d, src_gd[g], aux_d, small_d)
        o.copy_to_host_async()
        outs.append(o)

    # 4. fetch int8 z, apply the LN2 affine on host in f32
    z = np.stack([np.asarray(o) for o in outs], 2)          # [B,NCHUNK,G,SG,D] int8
    z = z.astype(np.float32) * (Z_CLIP / 127.0)
    out = z.reshape(B, SPAD, D)[:, :S]
    out = out * f32("ln2_w") + f32("ln2_b")
    return out
